# revision 5
# baseline (speedup 1.0000x reference)
"""HANLayer (2x GATConv + semantic attention) Trainium2 Bass kernel, 8 cores.

v3: node-sharded projection (1/8 per core) + AllGather of H12; host-packed
[em | emT] one-hot pair per chunk (single DMA); dv via matmul with emT;
scaled one-hots built on the Scalar engine (activation Copy with scale AP);
P3 transposes interleaved into the aggregation loop.
"""
import os
import sys

for _p in ("/opt/trn_rl_repo", "/root/.axon_site/_ro/trn_rl_repo"):
    if os.path.isdir(_p) and _p not in sys.path:
        sys.path.insert(0, _p)

import numpy as np
import ml_dtypes

import concourse.bacc as bacc
import concourse.bass as bass
import concourse.mybir as mybir
import concourse.tile as tile
from concourse import bass_utils
from concourse.masks import make_identity

F32 = mybir.dt.float32
BF16 = mybir.dt.bfloat16
I32 = mybir.dt.int32

N = 10000
E = 160000
IN_C = 512
OUT_C = 1024
NEG_SLOPE = 0.2
NCORES = 8
NPAD = 10240            # N padded to 80 blocks of 128
BPC = 10                # dst blocks per core
NODES_PER_CORE = 1280
TW = 2052               # H12 row: h1[0:1024] h2[1024:2048] [s1 d1 s2 d2]
P = 128

AddOp = mybir.AluOpType.add
SubOp = mybir.AluOpType.subtract
MulOp = mybir.AluOpType.mult
MaxOp = mybir.AluOpType.max
EqOp = mybir.AluOpType.is_equal
Byp = mybir.AluOpType.bypass


def _host_prep(edge_index):
    """Sort edges (plus self loops incl. pad nodes) by dst, chunk per core."""
    src = np.concatenate([edge_index[0].astype(np.int64),
                          np.arange(NPAD, dtype=np.int64)])
    dst = np.concatenate([edge_index[1].astype(np.int64),
                          np.arange(NPAD, dtype=np.int64)])
    order = np.argsort(dst, kind="stable")
    src_s = src[order]
    dst_s = dst[order]
    blk = dst_s // P
    counts = np.bincount(blk, minlength=NPAD // P)
    K = int(np.ceil(counts.max() / P))
    C = BPC * K
    src_idx = np.full((NCORES, P, C), 0x7FFFFFF, np.int32)
    embt = np.zeros((NCORES, C, P, 2 * P), ml_dtypes.bfloat16)
    bstart = np.searchsorted(blk, np.arange(NPAD // P + 1))
    for b in range(NPAD // P):
        core, bslot = divmod(b, BPC)
        lo, hi = bstart[b], bstart[b + 1]
        nb = hi - lo
        es = src_s[lo:hi]
        dl = (dst_s[lo:hi] - b * P).astype(np.int64)
        for c in range((nb + P - 1) // P):
            e0 = c * P
            e1 = min(e0 + P, nb)
            ci = bslot * K + c
            npts = e1 - e0
            src_idx[core, :npts, ci] = es[e0:e1]
            er = np.arange(npts)
            dlc = dl[e0:e1]
            embt[core, ci, er, dlc] = 1.0
            embt[core, ci, dlc, P + er] = 1.0
    return K, src_idx, embt


def _build_program(K, debug=False):
    C = BPC * K
    nc = bacc.Bacc("TRN2", target_bir_lowering=False, debug=False,
                   enable_asserts=False, num_devices=NCORES)

    # ---- inputs (replicated except XTL/SIDX/DLOC/MSK) ----
    XTL = nc.dram_tensor("XTL", [IN_C, NODES_PER_CORE], BF16,
                         kind="ExternalInput")
    W1 = nc.dram_tensor("W1", [IN_C, OUT_C], BF16, kind="ExternalInput")
    W2 = nc.dram_tensor("W2", [IN_C, OUT_C], BF16, kind="ExternalInput")
    W1TB = nc.dram_tensor("W1TB", [OUT_C, IN_C], BF16, kind="ExternalInput")
    W2TB = nc.dram_tensor("W2TB", [OUT_C, IN_C], BF16, kind="ExternalInput")
    A4 = nc.dram_tensor("A4", [OUT_C, 4], BF16, kind="ExternalInput")
    B1 = nc.dram_tensor("B1", [1, OUT_C], F32, kind="ExternalInput")
    B2 = nc.dram_tensor("B2", [1, OUT_C], F32, kind="ExternalInput")
    BP1R = nc.dram_tensor("BP1R", [1, OUT_C], BF16, kind="ExternalInput")
    PRA = nc.dram_tensor("PRA", [1, 1], F32, kind="ExternalInput")
    MSKB = nc.dram_tensor("MSKB", [P, 2], F32, kind="ExternalInput")
    WP1 = nc.dram_tensor("WP1", [OUT_C, OUT_C], BF16, kind="ExternalInput")
    WP2 = nc.dram_tensor("WP2", [OUT_C, OUT_C], BF16, kind="ExternalInput")
    SIDX = nc.dram_tensor("SIDX", [P, C], I32, kind="ExternalInput")
    EMBT = nc.dram_tensor("EMBT", [C, P, 2 * P], BF16, kind="ExternalInput")

    OUT = nc.dram_tensor("OUT", [NODES_PER_CORE, OUT_C], BF16,
                         kind="ExternalOutput")

    # ---- internal DRAM ----
    H12L = nc.dram_tensor("H12L", [NODES_PER_CORE, TW], BF16, kind="Internal")
    if debug:
        H12F = nc.dram_tensor("H12F", [NPAD, TW], BF16, kind="Internal")
        DBGH12F = nc.dram_tensor("DBGH12F", [NPAD, TW], BF16,
                                 kind="ExternalOutput")
        DBGH1 = nc.dram_tensor("DBGH1", [NODES_PER_CORE, OUT_C], F32,
                               kind="ExternalOutput")
        DBGH2 = nc.dram_tensor("DBGH2", [NODES_PER_CORE, OUT_C], F32,
                               kind="ExternalOutput")
        DBGD2 = nc.dram_tensor("DBGD2", [P, 2 * BPC], F32,
                               kind="ExternalOutput")
    else:
        H12F = nc.dram_tensor(
            "H12F", [NPAD, TW], BF16, kind="Internal",
            addr_space="Local" if os.environ.get("AG_LOCAL") else "Shared")
    ARIN = nc.dram_tensor("ARIN", [OUT_C], F32, kind="Internal")
    AROUT = nc.dram_tensor("AROUT", [OUT_C], F32, kind="Internal",
                           addr_space="Shared")
    ATTD = nc.dram_tensor("ATTD", [1, OUT_C], F32, kind="Internal")

    NKC = IN_C // P  # 4 k-chunks of input features

    with tile.TileContext(nc) as tc:
        with tc.tile_pool(name="persist", bufs=1) as pp:
            b1b = pp.tile([P, OUT_C], F32, tag="b1b")
            b2b = pp.tile([P, OUT_C], F32, tag="b2b")
            nc.sync.dma_start(b1b[:], B1.ap().to_broadcast((P, OUT_C)))
            nc.sync.dma_start(b2b[:], B2.ap().to_broadcast((P, OUT_C)))
            bp1r = pp.tile([1, OUT_C], BF16, tag="bp1r")
            nc.sync.dma_start(bp1r[:], BP1R.ap())
            pa_col = pp.tile([P, 1], F32, tag="pa_col")
            nc.sync.dma_start(pa_col[:], PRA.ap().to_broadcast((P, 1)))
            mskb = pp.tile([P, 2], F32, tag="mskb")
            nc.sync.dma_start(mskb[:], MSKB.ap())
            ones = pp.tile([P, 1], BF16, tag="ones")
            nc.vector.memset(ones[:], 1.0)
            ones_row = pp.tile([1, P], BF16, tag="ones_row")
            nc.vector.memset(ones_row[:], 1.0)
            identb = pp.tile([P, P], BF16, tag="identb")
            make_identity(nc, identb[:])
            sidx_t = pp.tile([P, C], I32, tag="sidx")
            nc.sync.dma_start(sidx_t[:], SIDX.ap())
            d2sb = pp.tile([P, 2 * BPC], BF16, tag="d2sb")
            # gat output stores (node-major, f32)
            h1st = pp.tile([P, BPC * OUT_C], F32, tag="h1st")
            h2st = pp.tile([P, BPC * OUT_C], F32, tag="h2st")
            # h-sum transposed (feature-major) for semantic attention
            htk = [pp.tile([P, NODES_PER_CORE], BF16, tag=f"htk{k}",
                           name=f"htk{k}") for k in range(8)]
            wp1k = [pp.tile([P, OUT_C], BF16, tag=f"wp1_{k}", name=f"wp1_{k}")
                    for k in range(8)]
            wp2k = [pp.tile([P, OUT_C], BF16, tag=f"wp2_{k}", name=f"wp2_{k}")
                    for k in range(8)]
            for k in range(8):
                nc.gpsimd.dma_start(wp1k[k][:],
                                    WP1.ap()[k * P:(k + 1) * P, :])
                nc.gpsimd.dma_start(wp2k[k][:],
                                    WP2.ap()[k * P:(k + 1) * P, :])

            # ================= P1: projection (node-sharded) =============
            with tc.tile_pool(name="p1sb", bufs=1) as sp, \
                 tc.tile_pool(name="p1ps", bufs=1, space="PSUM") as psp:
                rhsA_g = [sp.tile([P, OUT_C], BF16, tag=f"rhsA{g}",
                                  name=f"rhsA{g}") for g in range(NKC)]
                rhsB_g = [sp.tile([P, OUT_C], BF16, tag=f"rhsB{g}",
                                  name=f"rhsB{g}") for g in range(NKC)]
                wt_g = [sp.tile([P, 4], BF16, tag=f"wt{g}", name=f"wt{g}")
                        for g in range(NKC)]
                for g in range(NKC):
                    nc.sync.dma_start(rhsA_g[g][:],
                                      W1.ap()[g * P:(g + 1) * P, :])
                    nc.sync.dma_start(rhsB_g[g][:],
                                      W2.ap()[g * P:(g + 1) * P, :])
                w1tb = [sp.tile([P, IN_C], BF16, tag=f"w1tb{oc}",
                                name=f"w1tb{oc}") for oc in range(8)]
                w2tb = [sp.tile([P, IN_C], BF16, tag=f"w2tb{oc}",
                                name=f"w2tb{oc}") for oc in range(8)]
                a4t = [sp.tile([P, 4], BF16, tag=f"a4_{oc}", name=f"a4_{oc}")
                       for oc in range(8)]
                for oc in range(8):
                    nc.scalar.dma_start(w1tb[oc][:],
                                        W1TB.ap()[oc * P:(oc + 1) * P, :])
                    nc.scalar.dma_start(w2tb[oc][:],
                                        W2TB.ap()[oc * P:(oc + 1) * P, :])
                    nc.scalar.dma_start(a4t[oc][:],
                                        A4.ap()[oc * P:(oc + 1) * P, :])
                # wtilde: wt_g cols = [W1@as1, W1@ad1, W2@as2, W2@ad2]
                for g in range(NKC):
                    pwt1 = psp.tile([P, 2], F32, tag="pwt", bufs=4)
                    pwt2 = psp.tile([P, 2], F32, tag="pwt", bufs=4)
                    for oc in range(8):
                        nc.tensor.matmul(
                            pwt1[:], lhsT=w1tb[oc][:, g * P:(g + 1) * P],
                            rhs=a4t[oc][:, 0:2],
                            start=(oc == 0), stop=(oc == 7))
                        nc.tensor.matmul(
                            pwt2[:], lhsT=w2tb[oc][:, g * P:(g + 1) * P],
                            rhs=a4t[oc][:, 2:4],
                            start=(oc == 0), stop=(oc == 7))
                    nc.vector.tensor_copy(wt_g[g][:, 0:2], pwt1[:])
                    nc.vector.tensor_copy(wt_g[g][:, 2:4], pwt2[:])

                xg = [sp.tile([P, NODES_PER_CORE], BF16, tag=f"xg{g}",
                              name=f"xg{g}") for g in range(NKC)]
                for g in range(NKC):
                    nc.gpsimd.dma_start(xg[g][:],
                                        XTL.ap()[g * P:(g + 1) * P, :])
                groups = [(0, 512, 0), (512, 1024, 0), (0, 512, 1),
                          (512, 1024, 1), (0, 4, 2)]
                for l in range(BPC):
                    stg = sp.tile([P, TW], BF16, tag="stg", bufs=3)
                    for (lo, hi, which) in groups:
                        w = hi - lo
                        tag = "ph" if w == 512 else "pha"
                        ph = psp.tile([P, w], F32, tag=tag, bufs=2)
                        for g in range(NKC):
                            rhs = (rhsA_g, rhsB_g, wt_g)[which][g]
                            nc.tensor.matmul(ph[:],
                                             lhsT=xg[g][:, l * P:(l + 1) * P],
                                             rhs=rhs[:, lo:hi],
                                             start=(g == 0),
                                             stop=(g == NKC - 1))
                        slo = lo + which * OUT_C
                        nc.vector.tensor_copy(stg[:, slo:slo + w], ph[:])
                        if which == 2:
                            # d-cols [d1 d2] for local dst blocks, kept on-chip
                            nc.vector.tensor_copy(d2sb[:, 2 * l:2 * l + 2],
                                                  ph[:, 1:4:2])
                    nc.sync.dma_start(H12L.ap()[l * P:(l + 1) * P, :], stg[:])

            # gather the full projected table on every core
            nc.gpsimd.collective_compute(
                "AllGather", Byp,
                replica_groups=[list(range(NCORES))],
                ins=[H12L.ap().opt()], outs=[H12F.ap().opt()])

            # ================= P2: aggregation (dst-sharded) =============
            with tc.tile_pool(name="p2sb", bufs=1) as sp, \
                 tc.tile_pool(name="p2ps", bufs=1, space="PSUM") as psp:
                for b in range(BPC):
                    num1 = psp.tile([P, OUT_C], F32, tag="num1", bufs=1)
                    num2 = psp.tile([P, OUT_C], F32, tag="num2", bufs=1)
                    den1 = psp.tile([P, 1], F32, tag="den1", bufs=1)
                    den2 = psp.tile([P, 1], F32, tag="den2", bufs=1)
                    for c in range(K):
                        ci = b * K + c
                        st = (c == 0)
                        sp_ = (c == K - 1)
                        hg = sp.tile([P, TW], BF16, tag="hg", bufs=11)
                        if b == 0 and c < 10:
                            # warm up the rotating buffers: pad slots are
                            # skipped by the bounds check and would otherwise
                            # read uninitialized SBUF into the exp() chain
                            nc.vector.memset(hg[:], 0.0)
                        nc.gpsimd.indirect_dma_start(
                            out=hg[:], out_offset=None, in_=H12F.ap(),
                            in_offset=bass.IndirectOffsetOnAxis(
                                ap=sidx_t[:, ci:ci + 1], axis=0),
                            bounds_check=NPAD - 1, oob_is_err=False)
                        embt = sp.tile([P, 2 * P], BF16, tag="embt", bufs=10)
                        nc.sync.dma_start(embt[:], EMBT.ap()[ci])
                        dvp = psp.tile([P, 2], F32, tag="dvp", bufs=1)
                        nc.tensor.matmul(dvp[:], lhsT=embt[:, P:2 * P],
                                         rhs=d2sb[:, 2 * b:2 * b + 2],
                                         start=True, stop=True)
                        e2 = sp.tile([P, 2], F32, tag="e2", bufs=3)
                        nc.vector.tensor_tensor(
                            out=e2[:], in0=hg[:, 2048:2052:2], in1=dvp[:],
                            op=AddOp)
                        lr = sp.tile([P, 2], F32, tag="lr", bufs=3)
                        nc.vector.scalar_tensor_tensor(
                            out=lr[:], in0=e2[:], scalar=NEG_SLOPE,
                            in1=e2[:], op0=MulOp, op1=MaxOp)
                        al = sp.tile([P, 2], F32, tag="al", bufs=3)
                        nc.scalar.activation(al[:], lr[:],
                                             mybir.ActivationFunctionType.Exp)
                        a1m = sp.tile([P, P], BF16, tag="a1m", bufs=3)
                        a2m = sp.tile([P, P], BF16, tag="a2m", bufs=3)
                        nc.scalar.activation(
                            a1m[:], embt[:, 0:P],
                            mybir.ActivationFunctionType.Copy,
                            scale=al[:, 0:1])
                        nc.scalar.activation(
                            a2m[:], embt[:, 0:P],
                            mybir.ActivationFunctionType.Copy,
                            scale=al[:, 1:2])
                        nc.tensor.matmul(num1[:, 0:512], lhsT=a1m[:],
                                         rhs=hg[:, 0:512], start=st, stop=sp_)
                        nc.tensor.matmul(num1[:, 512:1024], lhsT=a1m[:],
                                         rhs=hg[:, 512:1024], start=st,
                                         stop=sp_)
                        nc.tensor.matmul(den1[:], lhsT=a1m[:],
                                         rhs=ones[:], start=st, stop=sp_)
                        nc.tensor.matmul(num2[:, 0:512], lhsT=a2m[:],
                                         rhs=hg[:, 1024:1536], start=st,
                                         stop=sp_)
                        nc.tensor.matmul(num2[:, 512:1024], lhsT=a2m[:],
                                         rhs=hg[:, 1536:2048], start=st,
                                         stop=sp_)
                        nc.tensor.matmul(den2[:], lhsT=a2m[:],
                                         rhs=ones[:], start=st, stop=sp_)
                    # drain block b: normalize, bias, prelu, transpose
                    hcols = slice(b * OUT_C, (b + 1) * OUT_C)
                    rden = sp.tile([P, 2], F32, tag="rden", bufs=2)
                    nc.vector.reciprocal(rden[:, 0:1], den1[:])
                    nc.vector.reciprocal(rden[:, 1:2], den2[:])
                    for gi, (numt, hst, bb) in enumerate(
                            [(num1, h1st, b1b), (num2, h2st, b2b)]):
                        tmp = sp.tile([P, OUT_C], F32, tag="gtmp", bufs=1)
                        nc.vector.scalar_tensor_tensor(
                            out=tmp[:], in0=numt[:],
                            scalar=rden[:, gi:gi + 1],
                            in1=bb[:], op0=MulOp, op1=AddOp)
                        # prelu(x) = max(a*x, x) for 0<=a<=1
                        nc.vector.scalar_tensor_tensor(
                            out=hst[:, hcols], in0=tmp[:],
                            scalar=pa_col[:, 0:1], in1=tmp[:],
                            op0=MulOp, op1=MaxOp)
                    hsb = sp.tile([P, OUT_C], BF16, tag="hsb", bufs=2)
                    nc.vector.tensor_tensor(out=hsb[:], in0=h1st[:, hcols],
                                            in1=h2st[:, hcols], op=AddOp)
                    # h1st := h1 - h2 (only the blend needs h1 from here on)
                    nc.vector.tensor_tensor(out=h1st[:, hcols],
                                            in0=h1st[:, hcols],
                                            in1=h2st[:, hcols], op=SubOp)
                    for oc in range(8):
                        tp = psp.tile([P, P], BF16, tag="trp", bufs=1)
                        nc.tensor.transpose(tp[:],
                                            hsb[:, oc * P:(oc + 1) * P],
                                            identb[:])
                        nc.vector.tensor_copy(
                            htk[oc][:, b * P:(b + 1) * P], tp[:])

            # ================= P3: semantic attention + blend ============
            with tc.tile_pool(name="p3sb", bufs=1) as sp, \
                 tc.tile_pool(name="p3ps", bufs=1, space="PSUM") as psp:
                # node-major: tps[n, o] = (h1+h2) @ Wp1 + bp1, per node block
                tsA = psp.tile([1, 512], F32, tag="tsA")
                tsB = psp.tile([1, 512], F32, tag="tsB")
                for nb in range(BPC):
                    ncols = slice(nb * P, (nb + 1) * P)
                    tps = psp.tile([P, OUT_C], F32, tag="tps", bufs=2)
                    for lo in (0, 512):
                        nc.tensor.matmul(tps[:, lo:lo + 512],
                                         lhsT=ones_row[:],
                                         rhs=bp1r[:, lo:lo + 512],
                                         start=True, stop=False)
                        for k in range(8):
                            nc.tensor.matmul(tps[:, lo:lo + 512],
                                             lhsT=htk[k][:, ncols],
                                             rhs=wp1k[k][:, lo:lo + 512],
                                             start=False, stop=(k == 7))
                    th = sp.tile([P, OUT_C], BF16, tag="th", bufs=2)
                    nc.scalar.activation(th[:], tps[:],
                                         mybir.ActivationFunctionType.Tanh)
                    if nb >= 8:
                        # mask pad nodes (>= local node 1040) on the last core
                        nc.vector.tensor_scalar_mul(
                            th[:], th[:], mskb[:, nb - 8:nb - 7])
                    nc.tensor.matmul(tsA[:], lhsT=ones[:], rhs=th[:, 0:512],
                                     start=(nb == 0), stop=(nb == BPC - 1))
                    nc.tensor.matmul(tsB[:], lhsT=ones[:], rhs=th[:, 512:1024],
                                     start=(nb == 0), stop=(nb == BPC - 1))
                tsum = sp.tile([1, OUT_C], F32, tag="tsum")
                nc.vector.tensor_copy(tsum[:, 0:512], tsA[:])
                nc.vector.tensor_copy(tsum[:, 512:1024], tsB[:])
                # allreduce node-sums of tanh over cores
                nc.sync.dma_start(bass.AP(ARIN, 0, [[1, 1], [1, OUT_C]]),
                                  tsum[:])
                nc.gpsimd.collective_compute(
                    "AllReduce", AddOp,
                    replica_groups=[list(range(NCORES))],
                    ins=[ARIN.ap().opt()], outs=[AROUT.ap().opt()])
                arview = [[1, P], [P, 8]]
                tbm = sp.tile([P, 8], F32, tag="tbm")
                nc.sync.dma_start(tbm[:], bass.AP(AROUT, 0, arview))
                tbn = sp.tile([P, 8], BF16, tag="tbn")
                nc.vector.tensor_scalar_mul(tbn[:], tbm[:], 1.0 / N)
                pw = psp.tile([1, OUT_C], F32, tag="pw")
                for k in range(8):
                    nc.tensor.matmul(pw[:, 0:512], lhsT=tbn[:, k:k + 1],
                                     rhs=wp2k[k][:, 0:512], start=(k == 0),
                                     stop=(k == 7))
                    nc.tensor.matmul(pw[:, 512:1024], lhsT=tbn[:, k:k + 1],
                                     rhs=wp2k[k][:, 512:1024], start=(k == 0),
                                     stop=(k == 7))
                et = sp.tile([1, OUT_C], F32, tag="et")
                esum = sp.tile([1, 1], F32, tag="esum")
                nc.scalar.activation(et[:], pw[:],
                                     mybir.ActivationFunctionType.Exp,
                                     accum_out=esum[:])
                rs = sp.tile([1, 1], F32, tag="rs")
                nc.vector.reciprocal(rs[:], esum[:])
                att1 = sp.tile([1, OUT_C], F32, tag="att1")
                nc.vector.tensor_scalar_mul(att1[:], et[:], rs[:, 0:1])
                nc.sync.dma_start(ATTD.ap(), att1[:])
                attb = sp.tile([P, OUT_C], F32, tag="attb")
                nc.sync.dma_start(attb[:],
                                  ATTD.ap().to_broadcast((P, OUT_C)))
                for b in range(BPC):
                    hcols = slice(b * OUT_C, (b + 1) * OUT_C)
                    m = sp.tile([P, OUT_C], BF16, tag="bm", bufs=2)
                    nc.vector.tensor_tensor(out=m[:], in0=h1st[:, hcols],
                                            in1=attb[:], op=MulOp)
                    o = sp.tile([P, OUT_C], BF16, tag="bo", bufs=2)
                    nc.vector.tensor_tensor(out=o[:], in0=m[:],
                                            in1=h2st[:, hcols], op=AddOp)
                    nc.sync.dma_start(OUT.ap()[b * P:(b + 1) * P, :], o[:])
                if debug:
                    for i in range(NPAD // P):
                        hcp = sp.tile([P, TW], BF16, tag="hcp", bufs=2)
                        nc.sync.dma_start(hcp[:],
                                          H12F.ap()[i * P:(i + 1) * P, :])
                        nc.sync.dma_start(DBGH12F.ap()[i * P:(i + 1) * P, :],
                                          hcp[:])
                    dbgd2 = sp.tile([P, 2 * BPC], F32, tag="dbgd2")
                    nc.vector.tensor_copy(dbgd2[:], d2sb[:])
                    nc.sync.dma_start(DBGD2.ap(), dbgd2[:])
                    for b in range(BPC):
                        hcols = slice(b * OUT_C, (b + 1) * OUT_C)
                        nc.sync.dma_start(
                            DBGH1.ap()[b * P:(b + 1) * P, :], h1st[:, hcols])
                        nc.sync.dma_start(
                            DBGH2.ap()[b * P:(b + 1) * P, :], h2st[:, hcols])

    nc.compile()
    return nc


_PROG_CACHE = {}


def _ensure_trace_support():
    """Install the missing antenv.axon_hooks NTFF shim so trace=True works."""
    import types
    try:
        from antenv import axon_hooks  # noqa: F401
        return True
    except ImportError:
        pass
    try:
        import antenv
        if "/root/.axon_site" not in sys.path:
            sys.path.append("/root/.axon_site")
        from trn_agent_boot.trn_boot import _ntff_profile_via_ctypes
        hook = _ntff_profile_via_ctypes("/opt/axon/libaxon_pjrt.so")
        if hook is None:
            return False
        mod = types.ModuleType("antenv.axon_hooks")
        mod._hook = hook
        mod.get_axon_ntff_profile_hook = lambda: mod._hook
        mod.set_axon_ntff_profile_hook = lambda h: setattr(mod, "_hook", h)
        sys.modules["antenv.axon_hooks"] = mod
        antenv.axon_hooks = mod
        bass_utils.upload_artifacts = lambda t: str(t)
        return True
    except Exception as e:  # noqa: BLE001
        print("trace support unavailable:", e)
        return False


def _get_program(K):
    if K not in _PROG_CACHE:
        _PROG_CACHE[K] = _build_program(K)
    return _PROG_CACHE[K]


def _run(inputs, trace=False, tmpdir=None, debug=False):
    x = np.asarray(inputs["x"], np.float32)
    edge_index = np.asarray(inputs["edge_index"])
    K, src_idx, embt = _host_prep(edge_index)
    if debug:
        nc = _build_program(K, debug=True)
    else:
        nc = _get_program(K)

    xpad = np.zeros((NPAD, IN_C), np.float32)
    xpad[:N] = x
    xT = np.ascontiguousarray(xpad.T).astype(ml_dtypes.bfloat16)
    W1f = np.ascontiguousarray(np.asarray(inputs["W1"], np.float32))
    W2f = np.ascontiguousarray(np.asarray(inputs["W2"], np.float32))
    A4 = np.ascontiguousarray(np.stack(
        [np.asarray(inputs["a_src1"], np.float32),
         np.asarray(inputs["a_dst1"], np.float32),
         np.asarray(inputs["a_src2"], np.float32),
         np.asarray(inputs["a_dst2"], np.float32)], axis=1))
    base = {
        "W1": W1f.astype(ml_dtypes.bfloat16),
        "W2": W2f.astype(ml_dtypes.bfloat16),
        "W1TB": np.ascontiguousarray(W1f.T).astype(ml_dtypes.bfloat16),
        "W2TB": np.ascontiguousarray(W2f.T).astype(ml_dtypes.bfloat16),
        "A4": A4.astype(ml_dtypes.bfloat16),
        "B1": np.asarray(inputs["b1"], np.float32).reshape(1, OUT_C),
        "B2": np.asarray(inputs["b2"], np.float32).reshape(1, OUT_C),
        "BP1R": np.asarray(inputs["bp1"], np.float32).reshape(1, OUT_C)
                  .astype(ml_dtypes.bfloat16),
        "PRA": np.asarray(inputs["prelu_a"], np.float32).reshape(1, 1),
        "WP1": np.ascontiguousarray(
            np.asarray(inputs["Wp1"], np.float32)).astype(ml_dtypes.bfloat16),
        "WP2": np.ascontiguousarray(
            np.asarray(inputs["Wp2"], np.float32)).astype(ml_dtypes.bfloat16),
    }
    in_maps = []
    for c in range(NCORES):
        m = dict(base)
        m["XTL"] = np.ascontiguousarray(
            xT[:, c * NODES_PER_CORE:(c + 1) * NODES_PER_CORE])
        mb = np.ones((P, 2), np.float32)
        if c == NCORES - 1:
            mb[16:, 0] = 0.0
            mb[:, 1] = 0.0
        m["MSKB"] = mb
        m["SIDX"] = np.ascontiguousarray(src_idx[c])
        m["EMBT"] = np.ascontiguousarray(embt[c])
        in_maps.append(m)

    if trace:
        trace = _ensure_trace_support()
    res = bass_utils.run_bass_kernel_spmd(
        nc, in_maps, core_ids=list(range(NCORES)), trace=trace,
        tmpdir=tmpdir)
    out = np.concatenate(
        [np.asarray(res.results[c]["OUT"], np.float32)
         for c in range(NCORES)], axis=0)[:N]
    if debug:
        return out, res
    return out, res.exec_time_ns


def kernel(**inputs):
    out, _ = _run(inputs, trace=False)
    return out
